# revision 19
# baseline (speedup 1.0000x reference)
"""Channel-attention module kernel for Trainium2 (8 NeuronCores, SPMD).

Computes, per sample b:
    q      = x[b].reshape(C, H*W)                    # [512, 4096]
    scores = q @ q.T                                 # [512, 512]
    attn   = softmax(scores, axis=-1)
    out[b] = gamma * (attn @ q) + x[b]

Sharding: data-parallel over B=16 across 8 cores (2 samples/core), gamma
replicated.  No cross-core communication.

Per-core per-sample pipeline (all shapes hardcoded for x=[16,512,64,64] f32):
  1. DMA x[s] in as 4 fp32 tiles [128c, 4096hw] (kept for the exact residual).
  2. Cast to bf16 (split across ScalarE/VectorE).
  3. DMA-xbar-transpose the bf16 tiles into QT [128hw, 32k, 512c]
     (contraction dim HW must live on partitions for the scores matmul).
  4. Scores: upper-triangle blocks only via PE matmul (bf16, fp32 PSUM);
     the strictly-lower blocks are filled by PE-transposing the staged
     upper off-diagonal blocks (S is symmetric).
  5. Row softmax: reduce_max (negated) -> ACT exp with per-partition bias,
     row-sum for free via accum_out; normalization is NOT applied to e --
     it is folded into the epilogue per-row scale gamma/sum[c].
  6. e is xbar-transposed to eT so that attn^T tiles are direct slices.
  7. out2' = e @ q via PE matmul (lhsT = eT slices, rhs = natural bf16 x).
  8. Epilogue: one fused DVE scalar_tensor_tensor:
         out = (psum * (gamma/sum[c])) + x_fp32
     then DMA to DRAM.  With gamma == 0 the output equals x bit-exactly.
"""

import numpy as np

import concourse.bass as bass
import concourse.tile as tile
from concourse import mybir
from concourse.bass_utils import run_bass_kernel_spmd
from concourse.masks import make_identity
from concourse.bass import _add_dep_helper

F32 = mybir.dt.float32
BF16 = mybir.dt.bfloat16
AF = mybir.ActivationFunctionType
ALU = mybir.AluOpType

P = 128          # partitions
C = 512          # channels
HW = 4096        # H*W
NB = C // P      # 4 channel blocks
NK = HW // P     # 32 hw chunks
S = 2            # samples per core
N_CORES = 8


def _split_xpose_waits(nc: bass.Bass) -> None:
    """The DMA_DIRECT2D_XPOSE instruction supports only one sync-wait slot.

    Tile can attach several (producer sem + xbar-mode serialization).  Move
    every wait onto standalone InstEventSemaphore fences in the same engine
    stream immediately before the transpose — the issuing sequencer executes
    them in program order, so semantics are identical.
    """
    for bb in nc.main_func.blocks:
        insts = bb.instructions
        out = []
        for ins in insts:
            si = getattr(ins, "sync_info", None)
            cap = 0 if isinstance(ins, mybir.InstDmaTransposeAnt) else 1
            if isinstance(ins, mybir.InstEventSemaphore):
                cap = None
            if cap is not None and si is not None and len(si.on_wait) > cap:
                moved = si.on_wait[: len(si.on_wait) - cap]
                kept = si.on_wait[len(si.on_wait) - cap :]
                for wi, w in enumerate(moved):
                    fence = mybir.InstEventSemaphore(
                        name=f"{ins.name}-prewait{wi}",
                        engine=ins.engine,
                        ins=[],
                        outs=[],
                        sync_info=mybir.SyncInfo(on_wait=[w], on_update=[]),
                    )
                    out.append(fence)
                ins.sync_info = mybir.SyncInfo(
                    on_wait=list(kept), on_update=list(si.on_update)
                )
            out.append(ins)
        insts[:] = out


def build_nc() -> bass.Bass:
    nc = bass.Bass()
    x_d = nc.declare_dram_parameter("x", [S, C, HW], F32, isOutput=False)
    g_d = nc.declare_dram_parameter("gamma", [1], F32, isOutput=False)
    o_d = nc.declare_dram_parameter("out", [S, C, HW], F32, isOutput=True)

    with tile.TileContext(nc) as tc:
        with (
            tc.tile_pool(name="xepipool", bufs=6) as xepipool,
            tc.tile_pool(name="xbfpool", bufs=8) as xbfpool,
            tc.tile_pool(name="qtpool", bufs=2) as qtpool,
            tc.tile_pool(name="epool", bufs=4) as epool,
            tc.tile_pool(name="etpool", bufs=2) as etpool,
            tc.tile_pool(name="outpool", bufs=3) as outpool,
            tc.tile_pool(name="stagepool", bufs=8) as stagepool,
            tc.tile_pool(name="statpool", bufs=24) as statpool,
            tc.tile_pool(name="singles", bufs=1) as singles,
            tc.tile_pool(name="spsum", bufs=2, space="PSUM") as spsum,
            tc.tile_pool(name="opsum", bufs=2, space="PSUM") as opsum,
            tc.tile_pool(name="tpsum", bufs=2, space="PSUM") as tpsum,
        ):
            # --- constants ---
            gamma_bc = singles.tile([P, 1], F32)
            nc.gpsimd.dma_start(out=gamma_bc, in_=g_d[:].to_broadcast((P, 1)))
            ident = singles.tile([P, P], F32)
            make_identity(nc, ident)
            ident_bf = singles.tile([P, P], BF16)
            make_identity(nc, ident_bf)

            prev_last_xpose = None
            for s in range(S):
                # --- load + cast ---
                xbf = []
                qt = qtpool.tile([P, NK, C], BF16, tag="qt", name=f"qt_{s}")
                H2 = HW // 2
                K2 = NK // 2
                loads = []
                for ib in range(NB):
                    xbf.append(
                        xbfpool.tile([P, HW], BF16, tag="xbf", name=f"xbf_{s}_{ib}")
                    )
                    ld = nc.gpsimd.dma_start(
                        out=xbf[ib], in_=x_d[s, P * ib : P * (ib + 1), :]
                    )
                    if prev_last_xpose is not None:
                        _add_dep_helper(
                            ld.ins, prev_last_xpose.ins, sync=True,
                            reason="copies wait for prior xpose batch",
                        )
                    loads.append(ld)
                qt_xposes = []
                for ib in range(NB):
                    xp = nc.scalar.dma_start_transpose(
                        qt[:, :, P * ib : P * (ib + 1)], xbf[ib]
                    )
                    if qt_xposes:
                        _add_dep_helper(
                            xp.ins, qt_xposes[-1].ins, sync=True,
                            reason="batch xposes together",
                        )
                    qt_xposes.append(xp)
                prev_last_xpose = qt_xposes[-1]

                # --- scores (upper triangle) + softmax pieces ---
                stage = {}
                e_tiles = []
                sg = []
                et = etpool.tile([P, NB, C], BF16, tag="et", name=f"et_{s}")
                for ib in range(NB):
                    ps = spsum.tile([P, C], F32, tag="ps", name=f"ps_{s}_{ib}")
                    for k in range(NK):
                        nc.tensor.matmul(
                            ps[:, P * ib :],
                            lhsT=qt[:, k, P * ib : P * (ib + 1)],
                            rhs=qt[:, k, P * ib :],
                            start=(k == 0),
                            stop=(k == NK - 1),
                        )
                    # stage upper off-diagonal blocks for later mirror fills
                    for j in range(ib + 1, NB):
                        st = stagepool.tile(
                            [P, P], F32, tag="sst", name=f"sst_{s}_{ib}_{j}"
                        )
                        nc.vector.tensor_copy(st, ps[:, P * j : P * (j + 1)])
                        stage[(ib, j)] = st
                    # mirror lower blocks: S(ib,j) = S(j,ib)^T
                    for j in range(ib):
                        nc.tensor.transpose(
                            ps[:, P * j : P * (j + 1)], stage[(j, ib)], ident
                        )

                    negm = statpool.tile([P, 1], F32, tag="negm", name=f"negm_{s}_{ib}")
                    nc.vector.tensor_reduce(
                        negm, ps, axis=mybir.AxisListType.X, op=ALU.max, negate=True
                    )
                    e_i = epool.tile([P, C], BF16, tag="e", name=f"e_{s}_{ib}")
                    ssum = statpool.tile([P, 1], F32, tag="ssum", name=f"ssum_{s}_{ib}")
                    nc.scalar.activation(
                        e_i, ps, AF.Exp, bias=negm, accum_out=ssum
                    )
                    inv = statpool.tile([P, 1], F32, tag="inv", name=f"inv_{s}_{ib}")
                    nc.vector.reciprocal(inv, ssum)
                    sg_i = statpool.tile([P, 1], F32, tag="sg", name=f"sg_{s}_{ib}")
                    nc.vector.tensor_tensor(sg_i, inv, gamma_bc, ALU.mult)
                    sg.append(sg_i)
                    e_tiles.append(e_i)
                    # eT via PE transpose (stays off the xbar DMA domain)
                    tp = tpsum.tile([P, C], BF16, tag="tp", name=f"tp_{s}_{ib}")
                    for kk in range(NB):
                        nc.tensor.transpose(
                            tp[:, P * kk : P * (kk + 1)],
                            e_i[:, P * kk : P * (kk + 1)],
                            ident_bf,
                        )
                    nc.scalar.activation(
                        et[:, :, P * ib : P * (ib + 1)],
                        tp.rearrange("p (a b) -> p a b", a=NB),
                        AF.Copy,
                    )

                # --- out2' = e @ q, fused epilogue, store ---
                for ib in range(NB):
                    for jh in range(4):
                        po = opsum.tile([P, 1024], F32, tag="po", name=f"po_{s}_{ib}_{jh}")
                        for kk in range(NB):
                            for j2 in range(2):
                                nc.tensor.matmul(
                                    po[:, 512 * j2 : 512 * (j2 + 1)],
                                    lhsT=et[:, kk, P * ib : P * (ib + 1)],
                                    rhs=xbf[kk][
                                        :, 1024 * jh + 512 * j2 : 1024 * jh + 512 * (j2 + 1)
                                    ],
                                    start=(kk == 0),
                                    stop=(kk == NB - 1),
                                )
                        xe = xepipool.tile(
                            [P, 1024], F32, tag="xe", name=f"xe_{s}_{ib}_{jh}"
                        )
                        xl = nc.sync.dma_start(
                            out=xe,
                            in_=x_d[
                                s, P * ib : P * (ib + 1), 1024 * jh : 1024 * (jh + 1)
                            ],
                        )
                        _add_dep_helper(
                            xl.ins, prev_last_xpose.ins, sync=True,
                            reason="epilogue loads after xpose batch",
                        )
                        ot = outpool.tile([P, 1024], F32, tag="ot", name=f"ot_{s}_{ib}_{jh}")
                        nc.vector.scalar_tensor_tensor(
                            ot,
                            in0=po,
                            scalar=sg[ib],
                            in1=xe,
                            op0=ALU.mult,
                            op1=ALU.add,
                        )
                        st_eng = nc.sync if jh % 2 == 0 else nc.gpsimd
                        st_eng.dma_start(
                            out=o_d[
                                s, P * ib : P * (ib + 1), 1024 * jh : 1024 * (jh + 1)
                            ],
                            in_=ot,
                        )
    _split_xpose_waits(nc)
    return nc


_NC_CACHE = None


def _get_nc():
    global _NC_CACHE
    if _NC_CACHE is None:
        _NC_CACHE = build_nc()
    return _NC_CACHE


def kernel(x, gamma):
    x = np.ascontiguousarray(np.asarray(x, dtype=np.float32))
    gamma = np.ascontiguousarray(np.asarray(gamma, dtype=np.float32)).reshape(1)
    B, Cc, H, W = x.shape
    xr = x.reshape(B, Cc, H * W)
    per = B // N_CORES
    nc = _get_nc()
    in_maps = [
        {"x": np.ascontiguousarray(xr[per * i : per * (i + 1)]), "gamma": gamma}
        for i in range(N_CORES)
    ]
    res = run_bass_kernel_spmd(nc, in_maps, list(range(N_CORES)))
    out = np.concatenate([r["out"] for r in res.results], axis=0)
    return out.reshape(B, Cc, H, W).astype(np.float32)


if __name__ == "__main__":
    xs = np.random.randn(16, 512, 64, 64).astype(np.float32)
    gs = np.zeros(1, np.float32)
    o = kernel(xs, gs)
    print("max |out - x| =", np.abs(o - xs).max())
